# revision 1
# baseline (speedup 1.0000x reference)
"""DeepseekMoE Trainium2 kernel — routed 3-stage pipeline on 8 NeuronCores.

Stage A (data-parallel, 1024 tokens/core): gate computed with a true-fp32
  matmul (so top-2 selection matches the fp32 reference) producing the
  normalized top-2 combine weights, plus the shared-expert FFN.
Stage B (expert-parallel, one expert per core): 3-layer FFN over the tokens
  routed to that expert (host-gathered to a runtime-sized capacity), with
  the per-token combine weight applied on device.
Stage C (data-parallel): out = shared + contrib1 + contrib2 on device.

Expert matmuls run in float32r (fp22 multiply, fp32 accumulate). Eval-mode
BatchNorm is folded into the expert weights host-side (pure parameter
preprocessing). Host code between stages only moves data (gather/scatter by
the device-computed top-2 indices); all per-token arithmetic is on device.
"""
import numpy as np
import concourse.mybir as mybir
import concourse.tile as tile
from concourse import bacc
from concourse.bass_utils import run_bass_kernel_spmd

F32 = mybir.dt.float32
F32R = mybir.dt.float32r

N_TOKENS, D, H, O, E = 8192, 1024, 2048, 1024, 8
KD, KH, MH, MO = D // 128, H // 128, H // 128, O // 128
NEXP = 9  # 8 routed experts + shared (index 8)
EPS = 1e-5
BIG = 1e30
N_CORES = 8
TOK = N_TOKENS // N_CORES
Relu = mybir.ActivationFunctionType.Relu
Sigm = mybir.ActivationFunctionType.Sigmoid
Expf = mybir.ActivationFunctionType.Exp


# ---------------------------------------------------------------- host prep
def _fold_params(inp):
    """Fold eval-mode BN into the expert weights (host-side, O(weights))."""
    def tiles_kxm(V, KT, MT):
        return np.ascontiguousarray(
            V.reshape(KT, 128, MT, 128).transpose(2, 1, 0, 3))

    V1s, V2s, V3s, C1s, C2s, C3s = [], [], [], [], [], []
    for e in range(NEXP):
        if e < E:
            W1, b1 = inp['W1'][e], inp['b1'][e]
            g1, be1, m1, v1 = inp['g1'][e], inp['be1'][e], inp['m1'][e], inp['v1'][e]
            W2, b2 = inp['W2'][e], inp['b2'][e]
            g2, be2, m2, v2 = inp['g2'][e], inp['be2'][e], inp['m2'][e], inp['v2'][e]
            W3, b3 = inp['W3'][e], inp['b3'][e]
        else:
            W1, b1 = inp['sW1'], inp['sb1']
            g1, be1, m1, v1 = inp['sg1'], inp['sbe1'], inp['sm1'], inp['sv1']
            W2, b2 = inp['sW2'], inp['sb2']
            g2, be2, m2, v2 = inp['sg2'], inp['sbe2'], inp['sm2'], inp['sv2']
            W3, b3 = inp['sW3'], inp['sb3']
        s1 = g1 / np.sqrt(v1 + EPS); t1 = be1 - m1 * s1
        s2 = g2 / np.sqrt(v2 + EPS); t2 = be2 - m2 * s2
        V1 = W1.T.astype(np.float32)
        c1 = b1.astype(np.float32)
        V2 = (s1[:, None] * W2.T).astype(np.float32)
        c2 = (b2 + t1 @ W2.T).astype(np.float32)
        V3 = (s2[:, None] * W3.T).astype(np.float32)
        c3 = (b3 + t2 @ W3.T).astype(np.float32)
        V1s.append(tiles_kxm(V1, KD, MH))
        V2s.append(tiles_kxm(V2, KH, MH))
        V3s.append(tiles_kxm(V3, KH, MO))
        C1s.append(np.ascontiguousarray(c1.reshape(MH, 128).T))
        C2s.append(np.ascontiguousarray(c2.reshape(MH, 128).T))
        C3s.append(np.ascontiguousarray(c3.reshape(MO, 128).T))
    return (np.stack(V1s), np.stack(V2s), np.stack(V3s),
            np.stack(C1s), np.stack(C2s), np.stack(C3s))


# ------------------------------------------------------------ kernel builders
def _ffn3(nc, pools, xg, V1_ap, V2_ap, V3_ap, c1_sb, c2_sb, c3_sb, ntok, emit):
    """Feature-major 3-layer FFN on `ntok` tokens (multiple of 128).
    xg: SBUF [128, KD, ntok] f32r. emit(mi, nsl, psum) consumes L3 psum."""
    wpool, ps, apool = pools["w"], pools["ps"], pools["act"]
    nsls = []
    s = 0
    while ntok - s > 0:
        rest = ntok - s
        if rest > 512 and rest < 768:
            # avoid a <256 tail: f32r matmuls below 256 free-dim run at 1/4 rate
            w = rest - 256
        else:
            w = min(512, rest)
        nsls.append(slice(s, s + w))
        s += w
    a1 = apool.tile([128, KH, ntok], F32R, tag="a1", name="a1")
    for mi in range(MH):
        wt = wpool.tile([128, KD, 128], F32R, tag="w", name="wt1")
        nc.sync.dma_start(wt, V1_ap[mi])
        for nsl in nsls:
            nn = nsl.stop - nsl.start
            pp = ps.tile([128, 512], F32, tag="ps", name="pp1")[:, :nn]
            for ki in range(KD):
                nc.tensor.matmul(pp, wt[:, ki], xg[:, ki, nsl],
                                 start=(ki == 0), stop=(ki == KD - 1))
            nc.scalar.activation(a1[:, mi, nsl], pp, Relu,
                                 bias=c1_sb[:, mi:mi + 1], scale=1.0)
    a2 = apool.tile([128, KH, ntok], F32R, tag="a2", name="a2")
    for mi in range(MH):
        wta = wpool.tile([128, KD, 128], F32R, tag="w", name="wta")
        nc.sync.dma_start(wta, V2_ap[mi, :, :KD])
        wtb = wpool.tile([128, KD, 128], F32R, tag="w", name="wtb")
        nc.sync.dma_start(wtb, V2_ap[mi, :, KD:])
        for nsl in nsls:
            nn = nsl.stop - nsl.start
            pp = ps.tile([128, 512], F32, tag="ps", name="pp2")[:, :nn]
            for ki in range(KH):
                wt = wta if ki < KD else wtb
                nc.tensor.matmul(pp, wt[:, ki % KD], a1[:, ki, nsl],
                                 start=(ki == 0), stop=(ki == KH - 1))
            nc.scalar.activation(a2[:, mi, nsl], pp, Relu,
                                 bias=c2_sb[:, mi:mi + 1], scale=1.0)
    for mi in range(MO):
        wta = wpool.tile([128, KD, 128], F32R, tag="w", name="wta3")
        nc.sync.dma_start(wta, V3_ap[mi, :, :KD])
        wtb = wpool.tile([128, KD, 128], F32R, tag="w", name="wtb3")
        nc.sync.dma_start(wtb, V3_ap[mi, :, KD:])
        for nsl in nsls:
            nn = nsl.stop - nsl.start
            pp = ps.tile([128, 512], F32, tag="ps", name="pp3")[:, :nn]
            for ki in range(KH):
                wt = wta if ki < KD else wtb
                nc.tensor.matmul(pp, wt[:, ki % KD], a2[:, ki, nsl],
                                 start=(ki == 0), stop=(ki == KH - 1))
            emit(mi, nsl, pp)


def _build_kernel_A():
    """Gate (true fp32) + shared expert. Outputs wsum [TOK, E], shared [O, TOK]."""
    nc = bacc.Bacc("TRN2", target_bir_lowering=False, debug=False,
                   num_devices=N_CORES)
    xTr_d = nc.dram_tensor("xTr", [D, TOK], F32R, kind="ExternalInput")
    xT32_d = nc.dram_tensor("xT32", [D, TOK], F32, kind="ExternalInput")
    wg_d = nc.dram_tensor("WgT", [D, E], F32, kind="ExternalInput")
    V1_d = nc.dram_tensor("V1s", [MH, 128, KD, 128], F32R, kind="ExternalInput")
    V2_d = nc.dram_tensor("V2s", [MH, 128, KH, 128], F32R, kind="ExternalInput")
    V3_d = nc.dram_tensor("V3s", [MO, 128, KH, 128], F32R, kind="ExternalInput")
    C1_d = nc.dram_tensor("C1s", [128, MH], F32, kind="ExternalInput")
    C2_d = nc.dram_tensor("C2s", [128, MH], F32, kind="ExternalInput")
    C3_d = nc.dram_tensor("C3s", [128, MO], F32, kind="ExternalInput")
    wsum_d = nc.dram_tensor("wsum", [TOK, E], F32, kind="ExternalOutput")
    sh_d = nc.dram_tensor("shared", [O, TOK], F32, kind="ExternalOutput")

    TT = TOK // 128
    with tile.TileContext(nc) as tc:
        with tc.tile_pool(name="const", bufs=1) as cpool, \
             tc.tile_pool(name="acts", bufs=1) as apool, \
             tc.tile_pool(name="wts", bufs=4) as wpool, \
             tc.tile_pool(name="bias", bufs=1) as bpool, \
             tc.tile_pool(name="tmp", bufs=3) as tpool, \
             tc.tile_pool(name="gate", bufs=2) as gpool, \
             tc.tile_pool(name="ps", bufs=8, space="PSUM") as ps:
            xTr_sb = cpool.tile([128, KD, TOK], F32R)
            for _ki in range(KD):
                nc.sync.dma_start(xTr_sb[:, _ki], xTr_d.ap().rearrange(
                    "(k p) t -> p k t", p=128)[:, _ki])
            wg_sb = cpool.tile([128, KD, E], F32)
            nc.sync.dma_start(wg_sb, wg_d.ap().rearrange("(k p) e -> p k e", p=128))
            # xT32 (gate-only) shares its slot with a2 (FFN L2+)
            xT32_sb = apool.tile([128, KD, TOK], F32, tag="a2", name="xT32_sb")
            for _ki in range(KD):
                nc.sync.dma_start(xT32_sb[:, _ki], xT32_d.ap().rearrange(
                    "(k p) t -> p k t", p=128)[:, _ki])

            for ti in range(TT):
                tsl = slice(ti * 128, (ti + 1) * 128)
                pg = ps.tile([128, 512], F32, tag="ps", name="pg")[:, :E]
                for ki in range(KD):
                    nc.tensor.matmul(pg, xT32_sb[:, ki, tsl], wg_sb[:, ki],
                                     start=(ki == 0), stop=(ki == KD - 1))
                s = gpool.tile([128, E], F32)
                nc.vector.tensor_copy(s, pg)
                m1 = gpool.tile([128, 1], F32)
                nc.vector.tensor_reduce(m1, s, axis=mybir.AxisListType.X,
                                        op=mybir.AluOpType.max)
                nm1 = gpool.tile([128, 1], F32)
                nc.vector.tensor_scalar_mul(nm1, m1, -1.0)
                msk = gpool.tile([128, E], F32)
                nc.vector.tensor_tensor(msk, s, m1.to_broadcast((128, E)),
                                        op=mybir.AluOpType.is_equal)
                nc.vector.tensor_scalar_mul(msk, msk, -BIG)
                nc.vector.tensor_tensor(msk, s, msk, op=mybir.AluOpType.add)
                m2 = gpool.tile([128, 1], F32)
                nc.vector.tensor_reduce(m2, msk, axis=mybir.AxisListType.X,
                                        op=mybir.AluOpType.max)
                r = gpool.tile([128, E], F32)
                nc.scalar.activation(r, s, Expf, bias=nm1, scale=1.0)
                e2 = gpool.tile([128, 1], F32)
                nc.scalar.activation(e2, m2, Expf, bias=nm1, scale=1.0)
                den = gpool.tile([128, 1], F32)
                nc.vector.tensor_scalar_add(den, e2, 1.0)
                rec = gpool.tile([128, 1], F32)
                nc.vector.reciprocal(rec, den)
                ge = gpool.tile([128, E], F32)
                nc.vector.tensor_tensor(ge, s, m2.to_broadcast((128, E)),
                                        op=mybir.AluOpType.is_ge)
                w = gpool.tile([128, E], F32)
                nc.vector.tensor_tensor(w, r, ge, op=mybir.AluOpType.mult)
                nc.vector.tensor_scalar_mul(w, w, rec)
                nc.sync.dma_start(wsum_d.ap()[tsl], w)

            c1_sb = bpool.tile([128, MH], F32, name="c1_sb")
            nc.sync.dma_start(c1_sb, C1_d.ap())
            c2_sb = bpool.tile([128, MH], F32, name="c2_sb")
            nc.sync.dma_start(c2_sb, C2_d.ap())
            c3_sb = bpool.tile([128, MO], F32, name="c3_sb")
            nc.sync.dma_start(c3_sb, C3_d.ap())

            def emit(mi, nsl, pp):
                nn = nsl.stop - nsl.start
                sg = tpool.tile([128, 512], F32, name="sg")[:, :nn]
                nc.scalar.activation(sg, pp, Sigm,
                                     bias=c3_sb[:, mi:mi + 1], scale=1.0)
                nc.sync.dma_start(sh_d.ap()[mi * 128:(mi + 1) * 128, nsl], sg)

            pools = {"w": wpool, "ps": ps, "act": apool}
            _ffn3(nc, pools, xTr_sb, V1_d.ap(), V2_d.ap(), V3_d.ap(),
                  c1_sb, c2_sb, c3_sb, TOK, emit)
    nc.compile()
    return nc


def _build_kernel_B(chunks):
    """One expert per core on gathered tokens; output pre-weighted [O, cap]."""
    C = sum(chunks)
    nc = bacc.Bacc("TRN2", target_bir_lowering=False, debug=False,
                   num_devices=N_CORES)
    xg_d = nc.dram_tensor("xg", [D, C], F32R, kind="ExternalInput")
    wrow_d = nc.dram_tensor("wrow", [C], F32, kind="ExternalInput")
    V1_d = nc.dram_tensor("V1s", [MH, 128, KD, 128], F32R, kind="ExternalInput")
    V2_d = nc.dram_tensor("V2s", [MH, 128, KH, 128], F32R, kind="ExternalInput")
    V3_d = nc.dram_tensor("V3s", [MO, 128, KH, 128], F32R, kind="ExternalInput")
    C1_d = nc.dram_tensor("C1s", [128, MH], F32, kind="ExternalInput")
    C2_d = nc.dram_tensor("C2s", [128, MH], F32, kind="ExternalInput")
    C3_d = nc.dram_tensor("C3s", [128, MO], F32, kind="ExternalInput")
    outb_d = nc.dram_tensor("outb", [O, C], F32, kind="ExternalOutput")

    with tile.TileContext(nc) as tc:
        with tc.tile_pool(name="xgp", bufs=1) as xgpool, \
             tc.tile_pool(name="acts", bufs=1) as apool, \
             tc.tile_pool(name="wts", bufs=4) as wpool, \
             tc.tile_pool(name="bias", bufs=1) as bpool, \
             tc.tile_pool(name="wb", bufs=1) as wbpool, \
             tc.tile_pool(name="tmp", bufs=3) as tpool, \
             tc.tile_pool(name="ps", bufs=8, space="PSUM") as ps:
            c1_sb = bpool.tile([128, MH], F32, name="c1_sb")
            nc.sync.dma_start(c1_sb, C1_d.ap())
            c2_sb = bpool.tile([128, MH], F32, name="c2_sb")
            nc.sync.dma_start(c2_sb, C2_d.ap())
            c3_sb = bpool.tile([128, MO], F32, name="c3_sb")
            nc.sync.dma_start(c3_sb, C3_d.ap())
            pools = {"w": wpool, "ps": ps, "act": apool}

            off = 0
            mx = max(chunks)
            for ch in chunks:
                xg = xgpool.tile([128, KD, mx], F32R, tag="xg", name="xg")
                for _ki in range(KD):
                    nc.sync.dma_start(
                        xg[:, _ki, :ch],
                        xg_d.ap().rearrange("(k p) t -> p k t",
                                            p=128)[:, _ki, off:off + ch])
                wbc = wbpool.tile([128, mx], F32, tag="wbc", name="wbc")
                nc.sync.dma_start(
                    wbc[:, :ch],
                    wrow_d.ap()[None, off:off + ch].to_broadcast((128, ch)))

                def emit(mi, nsl, pp, off=off, wbc=wbc):
                    nn = nsl.stop - nsl.start
                    sg = tpool.tile([128, 512], F32, name="sg")[:, :nn]
                    nc.scalar.activation(sg, pp, Sigm,
                                         bias=c3_sb[:, mi:mi + 1], scale=1.0)
                    nc.vector.tensor_tensor(sg, sg, wbc[:, nsl],
                                            op=mybir.AluOpType.mult)
                    nc.sync.dma_start(
                        outb_d.ap()[mi * 128:(mi + 1) * 128,
                                    off + nsl.start:off + nsl.stop], sg)

                _ffn3(nc, pools, xg[:, :, :ch], V1_d.ap(), V2_d.ap(),
                      V3_d.ap(), c1_sb, c2_sb, c3_sb, ch, emit)
                off += ch
    nc.compile()
    return nc


def _build_kernel_C():
    """out = sharedT + cont1 + cont2, all token-major [TOK, O]."""
    nc = bacc.Bacc("TRN2", target_bir_lowering=False, debug=False,
                   num_devices=N_CORES)
    sh_d = nc.dram_tensor("sharedT", [TOK, O], F32, kind="ExternalInput")
    c1_d = nc.dram_tensor("cont1", [TOK, O], F32, kind="ExternalInput")
    c2_d = nc.dram_tensor("cont2", [TOK, O], F32, kind="ExternalInput")
    out_d = nc.dram_tensor("out", [TOK, O], F32, kind="ExternalOutput")
    with tile.TileContext(nc) as tc:
        with tc.tile_pool(name="sb", bufs=3) as sb:
            for ti in range(TOK // 128):
                tsl = slice(ti * 128, (ti + 1) * 128)
                a = sb.tile([128, O], F32, name="a")
                nc.sync.dma_start(a, sh_d.ap()[tsl])
                b = sb.tile([128, O], F32, name="b")
                nc.sync.dma_start(b, c1_d.ap()[tsl])
                c = sb.tile([128, O], F32, name="c")
                nc.sync.dma_start(c, c2_d.ap()[tsl])
                nc.vector.tensor_tensor(a, a, b, op=mybir.AluOpType.add)
                nc.vector.tensor_tensor(a, a, c, op=mybir.AluOpType.add)
                nc.sync.dma_start(out_d.ap()[tsl], a)
    nc.compile()
    return nc


# ------------------------------------------------------------------ host glue
def _route_from_wsum(wsum):
    """Top-2 experts per token from the device-computed combine weights."""
    n = wsum.shape[0]
    top2 = np.argpartition(-wsum, 2, axis=1)[:, :2]
    sel = np.zeros_like(wsum, dtype=bool)
    sel[np.arange(n)[:, None], top2] = True
    idx = [np.nonzero(sel[:, e])[0] for e in range(E)]
    counts = np.array([len(i) for i in idx])
    # exact capacity (token dim needs no alignment); chunks <=1152 for SBUF,
    # near-even so every matmul free-dim tile stays >=256
    cap = max(512, int(np.ceil(counts.max() / 8) * 8))
    n_chunks = max(1, -(-cap // 1152))
    base = cap // n_chunks // 8 * 8
    rem8 = (cap - base * n_chunks) // 8
    chunks = [base + 8] * rem8 + [base] * (n_chunks - rem8)
    return idx, counts, tuple(chunks), cap, sel


_CACHED = {}


def kernel(**inputs) -> np.ndarray:
    inp = {k: np.asarray(v) for k, v in inputs.items()}
    V1r, V2r, V3r, C1, C2, C3 = _fold_params(inp)
    x = inp['x'].astype(np.float32)
    WgT = np.ascontiguousarray(inp['Wg'].T.astype(np.float32))

    # ---- stage A: gate + shared expert (data-parallel over tokens) ----
    if "A" not in _CACHED:
        _CACHED["A"] = _build_kernel_A()
    ncA = _CACHED["A"]
    shA = dict(WgT=WgT, V1s=V1r[8], V2s=V2r[8], V3s=V3r[8],
               C1s=C1[8], C2s=C2[8], C3s=C3[8])
    mapsA = []
    for c in range(N_CORES):
        xT = np.ascontiguousarray(x[c * TOK:(c + 1) * TOK].T)
        m = dict(shA)
        m['xTr'] = xT
        m['xT32'] = xT
        mapsA.append(m)
    resA = run_bass_kernel_spmd(ncA, mapsA, core_ids=list(range(N_CORES)))
    wsum = np.concatenate([r["wsum"] for r in resA.results], axis=0)
    sharedA = [r["shared"] for r in resA.results]

    # ---- host dispatch: gather tokens per expert ----
    idx, counts, chunks, cap, sel = _route_from_wsum(wsum)

    # ---- stage B: expert-parallel FFN on gathered tokens ----
    if _CACHED.get("B_chunks") != chunks:
        _CACHED["B"] = _build_kernel_B(chunks)
        _CACHED["B_chunks"] = chunks
    ncB = _CACHED["B"]
    mapsB = []
    for e in range(E):
        cnt = counts[e]
        xg = np.zeros((D, cap), np.float32)
        xg[:, :cnt] = x[idx[e]].T
        wrow = np.zeros((cap,), np.float32)
        wrow[:cnt] = wsum[idx[e], e]
        mapsB.append(dict(xg=xg, wrow=wrow, V1s=V1r[e], V2s=V2r[e], V3s=V3r[e],
                          C1s=C1[e], C2s=C2[e], C3s=C3[e]))
    resB = run_bass_kernel_spmd(ncB, mapsB, core_ids=list(range(N_CORES)))
    outbs = [r["outb"] for r in resB.results]

    # ---- host combine alignment: scatter contributions back by token ----
    first_e = np.argmax(sel, axis=1)
    cont1 = np.zeros((N_TOKENS, O), np.float32)
    cont2 = np.zeros((N_TOKENS, O), np.float32)
    for e in range(E):
        toks = idx[e]
        outT = np.ascontiguousarray(outbs[e][:, :counts[e]].T)
        is_first = first_e[toks] == e
        cont1[toks[is_first]] = outT[is_first]
        cont2[toks[~is_first]] = outT[~is_first]

    # ---- stage C: final on-device sum ----
    if "C" not in _CACHED:
        _CACHED["C"] = _build_kernel_C()
    ncC = _CACHED["C"]
    mapsC = []
    for c in range(N_CORES):
        sl = slice(c * TOK, (c + 1) * TOK)
        mapsC.append(dict(sharedT=np.ascontiguousarray(sharedA[c].T),
                          cont1=cont1[sl], cont2=cont2[sl]))
    resC = run_bass_kernel_spmd(ncC, mapsC, core_ids=list(range(N_CORES)))
    out = np.concatenate([r["out"] for r in resC.results], axis=0)

    _CACHED["timing"] = [(ncA, mapsA), (ncB, mapsB), (ncC, mapsC)]
    return out.astype(np.float32)



# revision 2
# speedup vs baseline: 1.6001x; 1.6001x over previous
"""DeepseekMoE Trainium2 kernel — routed 3-launch pipeline on 8 NeuronCores.

All FFN matmuls run as fp8(e4m3) DoubleRow tensor ops (0.5 cycles/row, 256-wide
contraction per instruction) with per-operand hi/lo residual splits choosing a
precision tier per (token, expert-slot):
  R  (320 cyc/tok): x hi/lo, W hi/lo, a1/a2 single fp8  — high combine weight
  B1 (256 cyc/tok): like R but x single fp8             — mid weight
  P8 (128 cyc/tok): everything single fp8               — low weight (w2<=0.35)
The shared expert runs scheme F (R plus an a2 hi/lo split).  Weight hi/lo
splits are host-side parameter preprocessing; the only data-dependent splits
(x, shared a2) are computed on device.

Launch A (data-parallel): fp32 gate (top-2 via sigmoid identity
  w1 = sigmoid(s1 - s2)), device x hi/lo split, shared-expert layer 1.
Launch B (expert-parallel, one expert per core): 3-layer FFN over
  host-gathered tokens in three tier chunks; outputs weighted fp16.
Launch C (data-parallel): shared layers 2+3 and final combine
  out = shared + cont1 + cont2.
Host code between launches only moves data (gather/scatter/layout); all
per-token arithmetic is on device.
"""
import numpy as np
import ml_dtypes
import concourse.mybir as mybir
import concourse.tile as tile
from concourse import bacc
from concourse.bass_utils import run_bass_kernel_spmd

F32 = mybir.dt.float32
F16 = mybir.dt.float16
F8 = mybir.dt.float8e4
E4 = ml_dtypes.float8_e4m3
DR = mybir.MatmulPerfMode.DoubleRow
AL = mybir.AluOpType
Relu = mybir.ActivationFunctionType.Relu
Sigm = mybir.ActivationFunctionType.Sigmoid

N_TOKENS, D, H, O, E = 8192, 1024, 2048, 1024, 8
N_CORES, TOK = 8, 1024
KD, KH = D // 128, H // 128          # contraction 128-blocks
KDP, KHP = KD // 2, KH // 2          # DoubleRow k-pairs
MH, MO = H // 128, O // 128          # output 128-tiles
GT = TOK // 128                      # gate token tiles per core
EPS = 1e-5
SW, SA = 32.0, 8.0                   # weight / activation fp8 storage scales
BIG = 1e30
THR1, THR2 = 0.6, 0.35               # tier thresholds on combine weight


# ---------------------------------------------------------------- host prep
def _wlayout(V):
    """V [K, M] fp32 -> [MT, 128, KP, 2, 2, 128] e4m3 hi/lo DoubleRow layout.
    k = j*256 + ksub*128 + p ; m = mi*128 + mm ; dim4 = (hi, lo)."""
    Kd, Md = V.shape
    KP, MT = Kd // 256, Md // 128
    s = (V * SW).astype(np.float32)
    hi = s.astype(E4)
    lo = (s - hi.astype(np.float32)).astype(E4)
    out = np.empty((MT, 128, KP, 2, 2, 128), E4)
    out[..., 0, :] = hi.reshape(KP, 2, 128, MT, 128).transpose(3, 2, 0, 1, 4)
    out[..., 1, :] = lo.reshape(KP, 2, 128, MT, 128).transpose(3, 2, 0, 1, 4)
    return np.ascontiguousarray(out)


def _fold_params(inp):
    """Fold eval-mode BN into weights; emit fp8 hi/lo layouts + scaled biases."""
    out = []
    for e in range(E + 1):
        if e < E:
            W1, b1 = inp['W1'][e], inp['b1'][e]
            g1, be1, m1, v1 = inp['g1'][e], inp['be1'][e], inp['m1'][e], inp['v1'][e]
            W2, b2 = inp['W2'][e], inp['b2'][e]
            g2, be2, m2, v2 = inp['g2'][e], inp['be2'][e], inp['m2'][e], inp['v2'][e]
            W3, b3 = inp['W3'][e], inp['b3'][e]
        else:
            W1, b1 = inp['sW1'], inp['sb1']
            g1, be1, m1, v1 = inp['sg1'], inp['sbe1'], inp['sm1'], inp['sv1']
            W2, b2 = inp['sW2'], inp['sb2']
            g2, be2, m2, v2 = inp['sg2'], inp['sbe2'], inp['sm2'], inp['sv2']
            W3, b3 = inp['sW3'], inp['sb3']
        s1 = g1 / np.sqrt(v1 + EPS); t1 = be1 - m1 * s1
        s2 = g2 / np.sqrt(v2 + EPS); t2 = be2 - m2 * s2
        V1 = W1.T.astype(np.float32)
        V2 = (s1[:, None] * W2.T).astype(np.float32)
        V3 = (s2[:, None] * W3.T).astype(np.float32)
        c1 = (SA * b1).astype(np.float32)
        c2 = (SA * (b2 + t1 @ W2.T)).astype(np.float32)
        c3 = (b3 + t2 @ W3.T).astype(np.float32)
        out.append(dict(
            V1=_wlayout(V1), V2=_wlayout(V2), V3=_wlayout(V3),
            c1=np.ascontiguousarray(c1.reshape(MH, 128).T),
            c2=np.ascontiguousarray(c2.reshape(MH, 128).T),
            c3=np.ascontiguousarray(c3.reshape(MO, 128).T)))
    return out


# ------------------------------------------------------------ layer builder
def _run_layer(nc, wpool, ps, wdram, MT, KP, chunks, mov, emit, tag):
    """One FFN layer over token chunks.  chunks: [(off, size, mode)];
    mov(j, hl, nsl) -> moving AP [128, 2, nn]; emit(mi, nsl, psum)."""
    for mi in range(MT):
        wt = wpool.tile([128, KP, 2, 2, 128], F8, tag="wt", name=f"wt{tag}")
        nc.sync.dma_start(wt, wdram[mi])
        for (off, size, mode) in chunks:
            if size == 0:
                continue
            s = 0
            while s < size:
                nn = min(256, size - s)
                nsl = slice(off + s, off + s + nn)
                pp = ps.tile([128, 512], F32, tag="ps", name=f"pp{tag}")[:, :nn]
                seq = []
                for j in range(KP):
                    hi_st = wt[:, j, :, 0, :]
                    lo_st = wt[:, j, :, 1, :]
                    if mode == "p8":
                        seq.append((hi_st, mov(j, 0, nsl)))
                    elif mode == "ws":
                        mh = mov(j, 0, nsl)
                        seq.append((hi_st, mh))
                        seq.append((lo_st, mh))
                    else:  # both-split
                        mh, ml = mov(j, 0, nsl), mov(j, 1, nsl)
                        seq += [(hi_st, mh), (hi_st, ml), (lo_st, mh), (lo_st, ml)]
                for i, (st, mv) in enumerate(seq):
                    nc.tensor.matmul(pp, st, mv, start=(i == 0),
                                     stop=(i == len(seq) - 1), perf_mode=DR)
                emit(mi, nsl, pp)
                s += nn


# ------------------------------------------------------------ kernel builders
def _build_A():
    """Gate (fp32) + x hi/lo split + shared-expert layer 1."""
    nc = bacc.Bacc("TRN2", target_bir_lowering=False, debug=False,
                   num_devices=N_CORES)
    xT_d = nc.dram_tensor("xT", [D, TOK], F32, kind="ExternalInput")
    wg_d = nc.dram_tensor("WgT", [D, E], F32, kind="ExternalInput")
    V1_d = nc.dram_tensor("V1s", [MH, 128, KDP, 2, 2, 128], F8, kind="ExternalInput")
    C1_d = nc.dram_tensor("C1s", [128, MH], F32, kind="ExternalInput")
    ws_d = nc.dram_tensor("wsum", [128, GT * E], F32, kind="ExternalOutput")
    xhl_d = nc.dram_tensor("xhl", [128, KD, 2, TOK], F8, kind="ExternalOutput")
    a1s_d = nc.dram_tensor("a1s", [128, MH, TOK], F8, kind="ExternalOutput")

    with tile.TileContext(nc) as tc:
        with tc.tile_pool(name="const", bufs=1) as cpool, \
             tc.tile_pool(name="gate", bufs=1) as gpool, \
             tc.tile_pool(name="wts", bufs=4) as wpool, \
             tc.tile_pool(name="ps", bufs=8, space="PSUM") as ps:
            x32 = cpool.tile([128, KD, TOK], F32, name="x32")
            for kb in range(KD):
                nc.sync.dma_start(
                    x32[:, kb], xT_d.ap().rearrange("(k p) t -> p k t", p=128)[:, kb])
            wg = cpool.tile([128, KD, E], F32, name="wg")
            nc.sync.dma_start(wg, wg_d.ap().rearrange("(k p) e -> p k e", p=128))
            c1_sb = cpool.tile([128, MH], F32, name="c1_sb")
            nc.sync.dma_start(c1_sb, C1_d.ap())

            # x hi/lo split
            xhl = cpool.tile([128, KD, 2, TOK], F8, name="xhl")
            for kb in range(KD):
                nc.vector.tensor_copy(xhl[:, kb, 0], x32[:, kb])
                nc.vector.tensor_tensor(xhl[:, kb, 1], x32[:, kb], xhl[:, kb, 0],
                                        op=AL.subtract)
            nc.sync.dma_start(xhl_d.ap(), xhl)

            # gate scores -> batched top-2 sigmoid weights
            sg = gpool.tile([128, GT, E], F32, name="sg")
            for ti in range(GT):
                tsl = slice(ti * 128, (ti + 1) * 128)
                pg = ps.tile([128, 512], F32, tag="ps", name="pg")[:, :E]
                for kb in range(KD):
                    nc.tensor.matmul(pg, x32[:, kb, tsl], wg[:, kb],
                                     start=(kb == 0), stop=(kb == KD - 1))
                nc.vector.tensor_copy(sg[:, ti], pg)
            m1 = gpool.tile([128, GT, 1], F32, name="m1")
            nc.vector.tensor_reduce(m1, sg, axis=mybir.AxisListType.X, op=AL.max)
            msk1 = gpool.tile([128, GT, E], F32, name="msk1")
            nc.vector.tensor_tensor(msk1, sg, m1.to_broadcast((128, GT, E)),
                                    op=AL.is_equal)
            pen = gpool.tile([128, GT, E], F32, name="pen")
            nc.vector.tensor_scalar_mul(pen, msk1, -BIG)
            nc.vector.tensor_tensor(pen, sg, pen, op=AL.add)
            m2 = gpool.tile([128, GT, 1], F32, name="m2")
            nc.vector.tensor_reduce(m2, pen, axis=mybir.AxisListType.X, op=AL.max)
            dm = gpool.tile([128, GT, 1], F32, name="dm")
            nc.vector.tensor_tensor(dm, m1, m2, op=AL.subtract)
            w1 = gpool.tile([128, GT, 1], F32, name="w1")
            nc.scalar.activation(w1, dm, Sigm, bias=0.0, scale=1.0)
            msk2 = gpool.tile([128, GT, E], F32, name="msk2")
            nc.vector.tensor_tensor(msk2, pen, m2.to_broadcast((128, GT, E)),
                                    op=AL.is_equal)
            t1 = gpool.tile([128, GT, E], F32, name="t1")
            nc.vector.tensor_tensor(t1, msk1, w1.to_broadcast((128, GT, E)),
                                    op=AL.mult)
            w2 = gpool.tile([128, GT, 1], F32, name="w2")
            nc.vector.tensor_scalar(w2, w1, -1.0, 1.0, op0=AL.mult, op1=AL.add)
            t2 = gpool.tile([128, GT, E], F32, name="t2")
            nc.vector.tensor_tensor(t2, msk2, w2.to_broadcast((128, GT, E)),
                                    op=AL.mult)
            wsm = gpool.tile([128, GT, E], F32, name="wsm")
            nc.vector.tensor_tensor(wsm, t1, t2, op=AL.add)
            nc.sync.dma_start(ws_d.ap(), wsm.rearrange("p a b -> p (a b)"))

            # shared expert layer 1 (both-split)
            a1s = cpool.tile([128, MH, TOK], F8, name="a1s")

            def emit1(mi, nsl, pp):
                nc.scalar.activation(a1s[:, mi, nsl], pp, Relu,
                                     bias=c1_sb[:, mi:mi + 1], scale=SA / SW)

            _run_layer(nc, wpool, ps, V1_d.ap(), MH, KDP,
                       [(0, TOK, "bs")],
                       lambda j, hl, nsl: xhl[:, 2 * j:2 * j + 2, hl, nsl],
                       emit1, "1")
            nc.sync.dma_start(a1s_d.ap(), a1s)
    nc.compile()
    return nc


def _build_B(caps):
    """One expert per core over gathered tokens in tier chunks (R, B1, P8)."""
    capR, capB, capP = caps
    CT = capR + capB + capP
    nc = bacc.Bacc("TRN2", target_bir_lowering=False, debug=False,
                   num_devices=N_CORES)
    xg_d = nc.dram_tensor("xg", [128, KD, 2, CT], F8, kind="ExternalInput")
    wr_d = nc.dram_tensor("wrow", [CT], F16, kind="ExternalInput")
    V1_d = nc.dram_tensor("V1s", [MH, 128, KDP, 2, 2, 128], F8, kind="ExternalInput")
    V2_d = nc.dram_tensor("V2s", [MH, 128, KHP, 2, 2, 128], F8, kind="ExternalInput")
    V3_d = nc.dram_tensor("V3s", [MO, 128, KHP, 2, 2, 128], F8, kind="ExternalInput")
    C1_d = nc.dram_tensor("C1s", [128, MH], F32, kind="ExternalInput")
    C2_d = nc.dram_tensor("C2s", [128, MH], F32, kind="ExternalInput")
    C3_d = nc.dram_tensor("C3s", [128, MO], F32, kind="ExternalInput")
    outb_d = nc.dram_tensor("outb", [O, CT], F16, kind="ExternalOutput")

    ch1 = [(0, capR, "bs"), (capR, capB, "ws"), (capR + capB, capP, "p8")]
    ch23 = [(0, capR, "ws"), (capR, capB, "ws"), (capR + capB, capP, "p8")]

    with tile.TileContext(nc) as tc:
        with tc.tile_pool(name="const", bufs=1) as cpool, \
             tc.tile_pool(name="acts", bufs=1) as apool, \
             tc.tile_pool(name="wts", bufs=4) as wpool, \
             tc.tile_pool(name="tmp", bufs=4) as tpool, \
             tc.tile_pool(name="ps", bufs=8, space="PSUM") as ps:
            xg = cpool.tile([128, KD, 2, CT], F8, name="xg")
            nc.sync.dma_start(xg, xg_d.ap())
            wbc = cpool.tile([128, CT], F16, name="wbc")
            nc.sync.dma_start(wbc, wr_d.ap()[None, :].to_broadcast((128, CT)))
            c1_sb = cpool.tile([128, MH], F32, name="c1_sb")
            nc.sync.dma_start(c1_sb, C1_d.ap())
            c2_sb = cpool.tile([128, MH], F32, name="c2_sb")
            nc.sync.dma_start(c2_sb, C2_d.ap())
            c3_sb = cpool.tile([128, MO], F32, name="c3_sb")
            nc.sync.dma_start(c3_sb, C3_d.ap())

            a1 = apool.tile([128, KH, CT], F8, name="a1")
            a2 = apool.tile([128, KH, CT], F8, name="a2")

            def emit1(mi, nsl, pp):
                nc.scalar.activation(a1[:, mi, nsl], pp, Relu,
                                     bias=c1_sb[:, mi:mi + 1], scale=SA / SW)

            _run_layer(nc, wpool, ps, V1_d.ap(), MH, KDP, ch1,
                       lambda j, hl, nsl: xg[:, 2 * j:2 * j + 2, hl, nsl],
                       emit1, "1")

            def emit2(mi, nsl, pp):
                nc.scalar.activation(a2[:, mi, nsl], pp, Relu,
                                     bias=c2_sb[:, mi:mi + 1], scale=1.0 / SW)

            _run_layer(nc, wpool, ps, V2_d.ap(), MH, KHP, ch23,
                       lambda j, hl, nsl: a1[:, 2 * j:2 * j + 2, nsl],
                       emit2, "2")

            def emit3(mi, nsl, pp):
                nn = nsl.stop - nsl.start
                sg = tpool.tile([128, 256], F16, tag="sg", name="sg")[:, :nn]
                nc.scalar.activation(sg, pp, Sigm,
                                     bias=c3_sb[:, mi:mi + 1], scale=1.0 / (SA * SW))
                nc.vector.tensor_tensor(sg, sg, wbc[:, nsl], op=AL.mult)
                nc.sync.dma_start(outb_d.ap()[mi * 128:(mi + 1) * 128, nsl], sg)

            _run_layer(nc, wpool, ps, V3_d.ap(), MO, KHP, ch23,
                       lambda j, hl, nsl: a2[:, 2 * j:2 * j + 2, nsl],
                       emit3, "3")
    nc.compile()
    return nc


def _build_C():
    """Shared expert layers 2+3 (with a2 hi/lo split) + final combine."""
    nc = bacc.Bacc("TRN2", target_bir_lowering=False, debug=False,
                   num_devices=N_CORES)
    a1_d = nc.dram_tensor("a1s", [128, MH, TOK], F8, kind="ExternalInput")
    V2_d = nc.dram_tensor("V2s", [MH, 128, KHP, 2, 2, 128], F8, kind="ExternalInput")
    V3_d = nc.dram_tensor("V3s", [MO, 128, KHP, 2, 2, 128], F8, kind="ExternalInput")
    C2_d = nc.dram_tensor("C2s", [128, MH], F32, kind="ExternalInput")
    C3_d = nc.dram_tensor("C3s", [128, MO], F32, kind="ExternalInput")
    c1t_d = nc.dram_tensor("cont1", [O, TOK], F16, kind="ExternalInput")
    c2t_d = nc.dram_tensor("cont2", [O, TOK], F16, kind="ExternalInput")
    out_d = nc.dram_tensor("out", [O, TOK], F32, kind="ExternalOutput")

    with tile.TileContext(nc) as tc:
        with tc.tile_pool(name="const", bufs=1) as cpool, \
             tc.tile_pool(name="acts", bufs=1) as apool, \
             tc.tile_pool(name="wts", bufs=4) as wpool, \
             tc.tile_pool(name="tmp", bufs=4) as tpool, \
             tc.tile_pool(name="ps", bufs=8, space="PSUM") as ps:
            a1 = cpool.tile([128, MH, TOK], F8, name="a1")
            nc.sync.dma_start(a1, a1_d.ap())
            c2_sb = cpool.tile([128, MH], F32, name="c2_sb")
            nc.sync.dma_start(c2_sb, C2_d.ap())
            c3_sb = cpool.tile([128, MO], F32, name="c3_sb")
            nc.sync.dma_start(c3_sb, C3_d.ap())

            a2hl = apool.tile([128, KH, 2, TOK], F8, name="a2hl")

            def emit2(mi, nsl, pp):
                nn = nsl.stop - nsl.start
                t32 = tpool.tile([128, 256], F32, tag="t32", name="t32")[:, :nn]
                nc.scalar.activation(t32, pp, Relu,
                                     bias=c2_sb[:, mi:mi + 1], scale=1.0 / SW)
                nc.vector.tensor_copy(a2hl[:, mi, 0, nsl], t32)
                nc.vector.tensor_tensor(a2hl[:, mi, 1, nsl], t32,
                                        a2hl[:, mi, 0, nsl], op=AL.subtract)

            _run_layer(nc, wpool, ps, V2_d.ap(), MH, KHP,
                       [(0, TOK, "ws")],
                       lambda j, hl, nsl: a1[:, 2 * j:2 * j + 2, nsl],
                       emit2, "2")

            def emit3(mi, nsl, pp):
                nn = nsl.stop - nsl.start
                sg = tpool.tile([128, 256], F16, tag="sg", name="sg")[:, :nn]
                nc.scalar.activation(sg, pp, Sigm,
                                     bias=c3_sb[:, mi:mi + 1], scale=1.0 / (SA * SW))
                ct1 = tpool.tile([128, 256], F16, tag="ct1", name="ct1")[:, :nn]
                nc.sync.dma_start(ct1, c1t_d.ap()[mi * 128:(mi + 1) * 128, nsl])
                ct2 = tpool.tile([128, 256], F16, tag="ct2", name="ct2")[:, :nn]
                nc.sync.dma_start(ct2, c2t_d.ap()[mi * 128:(mi + 1) * 128, nsl])
                s1 = tpool.tile([128, 256], F16, tag="s1", name="s1")[:, :nn]
                nc.vector.tensor_tensor(s1, sg, ct1, op=AL.add)
                so = tpool.tile([128, 256], F32, tag="so", name="so")[:, :nn]
                nc.vector.tensor_tensor(so, s1, ct2, op=AL.add)
                nc.sync.dma_start(out_d.ap()[mi * 128:(mi + 1) * 128, nsl], so)

            _run_layer(nc, wpool, ps, V3_d.ap(), MO, KHP,
                       [(0, TOK, "bs")],
                       lambda j, hl, nsl: a2hl[:, 2 * j:2 * j + 2, hl, nsl],
                       emit3, "3")
    nc.compile()
    return nc


# ------------------------------------------------------------------ host glue
def _r64(n):
    return max(64, (int(n) + 63) // 64 * 64)


def _route(wsum):
    """Per-expert tier column lists from device gate weights."""
    n = wsum.shape[0]
    e1 = np.argmax(wsum, axis=1)
    w1 = wsum[np.arange(n), e1]
    ws2 = wsum.copy()
    ws2[np.arange(n), e1] = 0.0
    e2 = np.argmax(ws2, axis=1)
    w2 = ws2[np.arange(n), e2]
    tiers = []  # per expert: dict tier -> (tokens, weights, is_first)
    for e in range(E):
        f = e1 == e
        s = e2 == e
        tR = np.nonzero(f & (w1 > THR1))[0]
        tBf = np.nonzero(f & (w1 <= THR1))[0]
        tBs = np.nonzero(s & (w2 > THR2))[0]
        tP = np.nonzero(s & (w2 <= THR2))[0]
        tB = np.concatenate([tBf, tBs])
        fB = np.concatenate([np.ones(len(tBf), bool), np.zeros(len(tBs), bool)])
        tiers.append(dict(
            R=(tR, w1[tR], np.ones(len(tR), bool)),
            B=(tB, np.concatenate([w1[tBf], w2[tBs]]), fB),
            P=(tP, w2[tP], np.zeros(len(tP), bool))))
    capR = _r64(max(len(t["R"][0]) for t in tiers))
    capB = _r64(max(len(t["B"][0]) for t in tiers))
    capP = _r64(max(len(t["P"][0]) for t in tiers))
    return tiers, (capR, capB, capP)


_CACHED = {}


def kernel(**inputs) -> np.ndarray:
    inp = {k: np.asarray(v) for k, v in inputs.items()}
    folded = _fold_params(inp)
    x = inp['x'].astype(np.float32)
    WgT = np.ascontiguousarray(inp['Wg'].T.astype(np.float32))
    sh = folded[E]

    # ---- launch A: gate + x split + shared L1 ----
    if "A" not in _CACHED:
        _CACHED["A"] = _build_A()
    ncA = _CACHED["A"]
    mapsA = []
    for c in range(N_CORES):
        xT = np.ascontiguousarray(x[c * TOK:(c + 1) * TOK].T)
        mapsA.append(dict(xT=xT, WgT=WgT, V1s=sh["V1"], C1s=sh["c1"]))
    resA = run_bass_kernel_spmd(ncA, mapsA, core_ids=list(range(N_CORES)))
    wsum = np.concatenate(
        [r["wsum"].reshape(128, GT, E).transpose(1, 0, 2).reshape(TOK, E)
         for r in resA.results], axis=0)
    xcat = np.concatenate([r["xhl"] for r in resA.results], axis=-1)
    a1s = [r["a1s"] for r in resA.results]

    # ---- host dispatch ----
    tiers, caps = _route(wsum)
    capR, capB, capP = caps
    CT = capR + capB + capP

    if _CACHED.get("B_caps") != caps:
        _CACHED["B"] = _build_B(caps)
        _CACHED["B_caps"] = caps
    ncB = _CACHED["B"]
    mapsB = []
    colmaps = []
    for e in range(E):
        xg = np.zeros((128, KD, 2, CT), E4)
        wrow = np.zeros((CT,), np.float16)
        cols, toks, isf = [], [], []
        for key, off, cap in (("R", 0, capR), ("B", capR, capB),
                              ("P", capR + capB, capP)):
            tk, wv, ff = tiers[e][key]
            m = len(tk)
            if m:
                xg[:, :, :, off:off + m] = xcat[:, :, :, tk]
                wrow[off:off + m] = wv.astype(np.float16)
                cols.append(np.arange(off, off + m))
                toks.append(tk)
                isf.append(ff)
        colmaps.append((np.concatenate(cols), np.concatenate(toks),
                        np.concatenate(isf)))
        fe = folded[e]
        mapsB.append(dict(xg=xg, wrow=wrow, V1s=fe["V1"], V2s=fe["V2"],
                          V3s=fe["V3"], C1s=fe["c1"], C2s=fe["c2"], C3s=fe["c3"]))
    resB = run_bass_kernel_spmd(ncB, mapsB, core_ids=list(range(N_CORES)))

    # ---- host combine alignment (column scatter, channel-major) ----
    cont1 = np.zeros((O, N_TOKENS), np.float16)
    cont2 = np.zeros((O, N_TOKENS), np.float16)
    for e in range(E):
        cols, toks, isf = colmaps[e]
        ob = resB.results[e]["outb"]
        cont1[:, toks[isf]] = ob[:, cols[isf]]
        cont2[:, toks[~isf]] = ob[:, cols[~isf]]

    # ---- launch C: shared L2+L3 + combine ----
    if "C" not in _CACHED:
        _CACHED["C"] = _build_C()
    ncC = _CACHED["C"]
    mapsC = []
    for c in range(N_CORES):
        sl = slice(c * TOK, (c + 1) * TOK)
        mapsC.append(dict(a1s=a1s[c], V2s=sh["V2"], V3s=sh["V3"],
                          C2s=sh["c2"], C3s=sh["c3"],
                          cont1=np.ascontiguousarray(cont1[:, sl]),
                          cont2=np.ascontiguousarray(cont2[:, sl])))
    resC = run_bass_kernel_spmd(ncC, mapsC, core_ids=list(range(N_CORES)))
    out = np.concatenate([np.ascontiguousarray(r["out"].T)
                          for r in resC.results], axis=0)

    _CACHED["timing"] = [(ncA, mapsA), (ncB, mapsB), (ncC, mapsC)]
    return out.astype(np.float32)
